# revision 4
# baseline (speedup 1.0000x reference)
"""Trainium2 Bass kernel for nn_Model_27685359190754 (RetinaNet-style head + NMS).

Self-contained: conv heads run on 8 NeuronCores (SPMD, data-parallel canvases);
top-k/decode/NMS postprocessing is replicated bitwise-exactly on host.

Sharding: each image's work is split over 4 cores — 3 cores carry 22/21/21-row
strips of the 64x64 level-0 feature map (with conv halos), 1 core carries
levels 1-4 packed into the same canvas geometry. All 8 cores run an identical
program; per-core behavior comes only from input data (canvas + mask).
"""
import numpy as np

# ---------------------------------------------------------------------------
# constants from the model
# ---------------------------------------------------------------------------
STRIDES = [8, 16, 32, 64, 128]
SIZES = [64, 32, 16, 8, 4]
TOP_N = 1000
THRESHOLD = 0.05
NMS_THR = 0.5
DETECTIONS = 100
NA = 9
NC = 80

CELL_ANCHORS = np.array([
 [[-18.0,-8.0,25.0,15.0],[-23.7183,-11.1191,30.7183,18.1191],[-30.9228,-15.0488,37.9228,22.0488],[-12.0,-12.0,19.0,19.0],[-16.1587,-16.1587,23.1587,23.1587],[-21.3984,-21.3984,28.3984,28.3984],[-8.0,-20.0,15.0,27.0],[-11.1191,-26.2381,18.1191,33.2381],[-15.0488,-34.0976,22.0488,41.0976]],
 [[-38.0,-16.0,53.0,31.0],[-49.9564,-22.2381,64.9564,37.2381],[-65.0204,-30.0976,80.0204,45.0976],[-24.0,-24.0,39.0,39.0],[-32.3175,-32.3175,47.3175,47.3175],[-42.7968,-42.7968,57.7968,57.7968],[-14.0,-36.0,29.0,51.0],[-19.7183,-47.4365,34.7183,62.4365],[-26.9228,-61.8456,41.9228,76.8456]],
 [[-74.0,-28.0,105.0,59.0],[-97.3929,-39.4365,128.3929,70.4365],[-126.8661,-53.8456,157.8661,84.8456],[-48.0,-48.0,79.0,79.0],[-64.6349,-64.6349,95.6349,95.6349],[-85.5937,-85.5937,116.5937,116.5937],[-30.0,-76.0,61.0,107.0],[-41.9564,-99.9127,72.9564,130.9127],[-57.0204,-130.0409,88.0204,161.0409]],
 [[-150.0,-60.0,213.0,123.0],[-197.3056,-83.9127,260.3056,146.9127],[-256.907,-114.0409,319.907,177.0409],[-96.0,-96.0,159.0,159.0],[-129.2699,-129.2699,192.2699,192.2699],[-171.1873,-171.1873,234.1873,234.1873],[-58.0,-148.0,121.0,211.0],[-81.3929,-194.7858,144.3929,257.7858],[-110.8661,-253.7322,173.8661,316.7322]],
 [[-298.0,-116.0,425.0,243.0],[-392.0914,-162.7858,519.0914,289.7858],[-510.6392,-221.7322,637.6392,348.7322],[-192.0,-192.0,319.0,319.0],[-258.5398,-258.5398,385.5398,385.5398],[-342.3747,-342.3747,469.3747,469.3747],[-118.0,-300.0,245.0,427.0],[-165.3056,-394.6113,292.3056,521.6113],[-224.907,-513.814,351.907,640.814]]], dtype=np.float32)

# canvas geometry
CR, CC = 34, 66            # canvas rows / cols
CN = CR * CC               # 2244
OUT_ROWS = (1, 33)         # canvas rows where convs are computed
NOUT = (OUT_ROWS[1] - OUT_ROWS[0]) * CC   # 2112 output positions
# trunk/final chunking (rows per matmul chunk)
CHUNKS = [(1, 8), (8, 15), (15, 22), (22, 29), (29, 33)]

# L0 strips (3 per image): output rows
L0_STRIPS = [(0, 22), (22, 43), (43, 64)]
# packed placement for levels 1-4 on the canvas: (level, row0, col0) of the
# *data* cell (top-left), sizes from SIZES.
PACK = [(1, 1, 1), (2, 1, 35), (3, 19, 35), (4, 19, 47)]

_COMPILED = {}
LAST_EXEC_NS = None
LAST_RUN_S = None
LAST_POST_S = None


# ---------------------------------------------------------------------------
# host-side bitwise-exact postprocess (replicates reference decode+NMS,
# validated bitwise against the XLA CPU reference)
# ---------------------------------------------------------------------------
def _fma(a, b, c):
    return (np.asarray(a, np.float64) * np.asarray(b, np.float64)
            + np.asarray(c, np.float64)).astype(np.float32)

_P = [np.float32(v) for v in [1.9875691500E-4, 1.3981999507E-3, 8.3334519073E-3,
                              4.1665795894E-2, 1.6666665459E-1, 5.0000001201E-1]]
_LOG2E = np.float32(1.442695040888963407359924681001892137)
_C1 = np.float32(0.693359375)
_C2 = np.float32(-2.12194440e-4)

def xla_exp(x):
    x = np.asarray(x, np.float32)
    n = np.rint((x * _LOG2E).astype(np.float32)).astype(np.float32)
    r = _fma(n, np.full_like(x, -_C1), x)
    r = _fma(n, np.full_like(x, -_C2), r)
    y = np.full_like(r, _P[0])
    for c in _P[1:]:
        y = _fma(y, r, np.full_like(r, c))
    r2 = (r * r).astype(np.float32)
    y = (_fma(y, r2, r) + np.float32(1.0)).astype(np.float32)
    return np.ldexp(y, n.astype(np.int32)).astype(np.float32)

def xla_sigmoid(x):
    x = np.asarray(x, np.float32)
    e = xla_exp((-x).astype(np.float32))
    return (np.float32(1.0) / (e + np.float32(1.0)).astype(np.float32)).astype(np.float32)

def _delta2box_np(deltas, anchors, stride):
    deltas = deltas.astype(np.float32)
    anchors = anchors.astype(np.float32)
    awh = (anchors[:, 2:] - anchors[:, :2] + np.float32(1.0)).astype(np.float32)
    ctr = (anchors[:, :2] + (np.float32(0.5) * awh).astype(np.float32)).astype(np.float32)
    pctr = ((deltas[:, :2] * awh).astype(np.float32) + ctr).astype(np.float32)
    pwh = (xla_exp(deltas[:, 2:]) * awh).astype(np.float32)
    M = np.float32(511.0)  # w*stride - 1 == 511 for every level
    half = (np.float32(0.5) * pwh).astype(np.float32)
    lo = np.clip((pctr - half).astype(np.float32), np.float32(0.0), M).astype(np.float32)
    hi = np.clip(((pctr + half).astype(np.float32) - np.float32(1.0)).astype(np.float32),
                 np.float32(0.0), M).astype(np.float32)
    return np.concatenate([lo, hi], axis=1).astype(np.float32)

def _decode_level_np(logits, box, stride, anchors):
    AC, H, W = logits.shape
    cm = xla_sigmoid(logits.reshape(-1))
    idx = np.argsort(-cm, kind='stable')[:TOP_N]
    scores = cm[idx]
    cls = (idx // (W * H)) % NC
    x = idx % W
    y = (idx // W) % H
    a = idx // (NC * H * W)
    deltas = box.reshape(NA, 4, H, W)[a, :, y, x]
    grid = (np.stack([x, y, x, y], axis=1).astype(np.float32) * np.float32(stride)
            + anchors[a]).astype(np.float32)
    boxes = _delta2box_np(deltas, grid, stride)
    valid = scores >= np.float32(THRESHOLD)
    return (np.where(valid, scores, np.float32(0.0)).astype(np.float32),
            np.where(valid[:, None], boxes, np.float32(0.0)).astype(np.float32),
            np.where(valid, cls.astype(np.float32), np.float32(0.0)).astype(np.float32))

def _nms_image_np(s, b, c):
    order = np.argsort(-s, kind='stable')
    so, bo, co = s[order], b[order], c[order]
    areas = ((bo[:, 2] - bo[:, 0] + np.float32(1.0)) *
             (bo[:, 3] - bo[:, 1] + np.float32(1.0))).astype(np.float32)
    valid = so > 0
    os_ = np.zeros(DETECTIONS, np.float32)
    ob_ = np.zeros((DETECTIONS, 4), np.float32)
    oc_ = np.zeros(DETECTIONS, np.float32)
    for i in range(DETECTIONS):
        if not valid.any():
            break
        idx = int(np.argmax(valid))
        bi, si, ci, ai = bo[idx], so[idx], co[idx], areas[idx]
        xy1 = np.maximum(bo[:, :2], bi[:2])
        xy2 = np.minimum(bo[:, 2:], bi[2:])
        iw = np.clip((xy2 - xy1 + np.float32(1.0)).astype(np.float32), 0.0, None)
        inter = (iw[:, 0] * iw[:, 1]).astype(np.float32)
        iou = (inter / ((areas + ai).astype(np.float32) - inter).astype(np.float32)).astype(np.float32)
        sup = (so <= si) & (iou > np.float32(NMS_THR)) & (co == ci)
        valid = valid & ~sup
        os_[i] = si
        ob_[i] = bi
        oc_[i] = ci
    return os_, ob_, oc_

def _postprocess(cls_logit_maps, box_maps):
    Bn = cls_logit_maps[0].shape[0]
    out_s = np.zeros((Bn, DETECTIONS), np.float32)
    out_b = np.zeros((Bn, DETECTIONS, 4), np.float32)
    out_c = np.zeros((Bn, DETECTIONS), np.float32)
    for img in range(Bn):
        ss, bs, cs = [], [], []
        for lvl in range(5):
            s, b, c = _decode_level_np(cls_logit_maps[lvl][img], box_maps[lvl][img],
                                       STRIDES[lvl], CELL_ANCHORS[lvl].astype(np.float32))
            ss.append(s); bs.append(b); cs.append(c)
        os_, ob_, oc_ = _nms_image_np(np.concatenate(ss), np.concatenate(bs),
                                      np.concatenate(cs))
        out_s[img] = os_; out_b[img] = ob_; out_c[img] = oc_
    return out_s, out_b, out_c


# ---------------------------------------------------------------------------
# device kernel: generic canvas conv machine
# ---------------------------------------------------------------------------
def _build_kernel():
    import concourse.bass as bass
    import concourse.mybir as mybir
    from concourse import bacc
    from concourse.tile import TileContext

    dt = mybir.dt
    nc = bacc.Bacc()

    xin = nc.dram_tensor("xin", [2, 128, CN], dt.float32, kind="ExternalInput")
    msk = nc.dram_tensor("msk", [128, CN], dt.float32, kind="ExternalInput")
    # trunk weights: [head, layer, tap, cin_half, 128, 256]
    wtr = nc.dram_tensor("wtr", [2, 4, 9, 2, 128, 256], dt.float32, kind="ExternalInput")
    btr = nc.dram_tensor("btr", [2, 4, 256], dt.float32, kind="ExternalInput")
    wcf = nc.dram_tensor("wcf", [9, 2, 128, 720], dt.float32, kind="ExternalInput")
    bcf = nc.dram_tensor("bcf", [720], dt.float32, kind="ExternalInput")
    wbf = nc.dram_tensor("wbf", [9, 2, 128, 36], dt.float32, kind="ExternalInput")
    bbf = nc.dram_tensor("bbf", [36], dt.float32, kind="ExternalInput")
    cls_out = nc.dram_tensor("cls_out", [6, 120, NOUT], dt.float32, kind="ExternalOutput")
    box_out = nc.dram_tensor("box_out", [36, NOUT], dt.float32, kind="ExternalOutput")

    XB = CN + 2  # activation buffer: canvas + 1 pad element each side

    with TileContext(nc) as tc:
        with tc.tile_pool(name="acts", bufs=1) as acts, \
             tc.tile_pool(name="wts", bufs=2) as wts, \
             tc.tile_pool(name="wfin", bufs=1) as wfin, \
             tc.tile_pool(name="small", bufs=1) as small, \
             tc.tile_pool(name="stage", bufs=4) as stage, \
             tc.tile_pool(name="psum", bufs=4, space="PSUM") as pp:

            # mask
            mt = small.tile([128, CN], dt.float32, tag="mask")
            nc.sync.dma_start(mt[:], msk[:, :])

            # biases: trunk [128, 16] (head*8 + layer*2 + couthalf)
            btr_t = small.tile([128, 16], dt.float32, tag="btr")
            for h in range(2):
                for l in range(4):
                    for ch in range(2):
                        col = h * 8 + l * 2 + ch
                        nc.sync.dma_start(
                            btr_t[:, col:col + 1],
                            btr[h, l, ch * 128:(ch + 1) * 128].unsqueeze(1))
            bcf_t = small.tile([120, 6], dt.float32, tag="bcf")
            for t in range(6):
                nc.sync.dma_start(bcf_t[:, t:t + 1],
                                  bcf[t * 120:(t + 1) * 120].unsqueeze(1))
            bbf_t = small.tile([36, 1], dt.float32, tag="bbf")
            nc.sync.dma_start(bbf_t[:, 0:1], bbf[:].unsqueeze(1))

            # input canvas (kept for both heads)
            xin_t = []
            for half in range(2):
                t = acts.tile([128, XB], dt.float32, tag=f"xin{half}")
                nc.vector.memset(t[:], 0.0)
                nc.sync.dma_start(t[:, 1:1 + CN], xin[half])
                xin_t.append(t)

            # work buffers (ping/pong per half)
            work = []
            for i in range(4):
                t = acts.tile([128, XB], dt.float32, tag=f"work{i}")
                nc.vector.memset(t[:], 0.0)
                work.append(t)

            def trunk_layer(head, layer, src, dst):
                # load weights [128, 9*2*256] view: (tap, cih, cout)
                w = wts.tile([128, 9, 2, 256], dt.float32, tag="wtr")
                nc.sync.dma_start(w[:], wtr[head, layer].transpose([2, 0, 1, 3]))
                for (a, b) in CHUNKS:
                    n = (b - a) * CC
                    for coh in range(2):
                        ps = pp.tile([128, 512], dt.float32, tag="ps")
                        k = 0
                        for tap in range(9):
                            dy, dx = tap // 3 - 1, tap % 3 - 1
                            off = 1 + a * CC + dy * CC + dx
                            for cih in range(2):
                                nc.tensor.matmul(
                                    ps[:, :n],
                                    w[:, tap, cih, coh * 128:(coh + 1) * 128],
                                    src[cih][:, off:off + n],
                                    start=(k == 0), stop=(k == 17))
                                k += 1
                        # relu(psum + bias) then mask-multiply into dst
                        st = stage.tile([128, 512], dt.float32, tag="st")
                        bcol = head * 8 + layer * 2 + coh
                        nc.scalar.activation(st[:, :n], ps[:, :n],
                                             mybir.ActivationFunctionType.Relu,
                                             bias=btr_t[:, bcol:bcol + 1])
                        nc.vector.tensor_mul(dst[coh][:, 1 + a * CC:1 + a * CC + n],
                                             st[:, :n], mt[:, a * CC:a * CC + n])

            def final_layer(head, src):
                if head == 0:
                    wf = wfin.tile([128, 9, 2, 720], dt.float32, tag="wcf")
                    nc.sync.dma_start(wf[:], wcf[:].transpose([2, 0, 1, 3]))
                    tiles, tw, bias_t, outt = 6, 120, bcf_t, cls_out
                else:
                    wf = wfin.tile([128, 9, 2, 36], dt.float32, tag="wbf")
                    nc.sync.dma_start(wf[:], wbf[:].transpose([2, 0, 1, 3]))
                    tiles, tw, bias_t, outt = 1, 36, bbf_t, box_out
                for (a, b) in CHUNKS:
                    n = (b - a) * CC
                    pos0 = (a - 1) * CC
                    for t in range(tiles):
                        ps = pp.tile([128, 512], dt.float32, tag="ps")
                        k = 0
                        for tap in range(9):
                            dy, dx = tap // 3 - 1, tap % 3 - 1
                            off = 1 + a * CC + dy * CC + dx
                            for cih in range(2):
                                nc.tensor.matmul(
                                    ps[:tw, :n],
                                    wf[:, tap, cih, t * tw:(t + 1) * tw],
                                    src[cih][:, off:off + n],
                                    start=(k == 0), stop=(k == 17))
                                k += 1
                        st = stage.tile([128, 512], dt.float32, tag="st")
                        nc.scalar.activation(st[:tw, :n], ps[:tw, :n],
                                             mybir.ActivationFunctionType.Identity,
                                             bias=bias_t[:, t:t + 1] if tiles > 1 else bias_t[:, 0:1])
                        if head == 0:
                            nc.sync.dma_start(outt[t, :, pos0:pos0 + n], st[:tw, :n])
                        else:
                            nc.sync.dma_start(outt[:, pos0:pos0 + n], st[:tw, :n])

            for head in range(2):
                src = xin_t
                for layer in range(4):
                    dst = [work[(layer % 2) * 2], work[(layer % 2) * 2 + 1]]
                    trunk_layer(head, layer, src, dst)
                    src = dst
                final_layer(head, src)

    nc.finalize()
    return nc


def _prep_weights(inputs):
    """Host-side weight layout prep: [O,I,3,3] -> [9, 2, 128, O]."""
    def tr(w):
        O = w.shape[0]
        t = np.ascontiguousarray(w.transpose(2, 3, 1, 0).reshape(9, 2, 128, O),
                                 dtype=np.float32)
        return t
    wtr = np.stack([
        np.stack([tr(inputs['cls_w%d' % l]) for l in range(4)]),
        np.stack([tr(inputs['box_w%d' % l]) for l in range(4)]),
    ]).astype(np.float32)
    btr = np.stack([
        np.stack([inputs['cls_b%d' % l] for l in range(4)]),
        np.stack([inputs['box_b%d' % l] for l in range(4)]),
    ]).astype(np.float32)
    return (np.ascontiguousarray(wtr), np.ascontiguousarray(btr),
            tr(inputs['cls_w4']), inputs['cls_b4'].astype(np.float32),
            tr(inputs['box_w4']), inputs['box_b4'].astype(np.float32))


def _make_canvases(inputs):
    """Returns per-core (xin [2,128,CN], msk [128,CN]) and extraction info."""
    feats = [np.asarray(inputs['feat%d' % i], np.float32) for i in range(5)]
    cores = []
    for img in range(2):
        for k in range(3):
            r0, r1 = L0_STRIPS[k]
            a, b = max(0, r0 - 5), min(64, r1 + 5)
            canvas = np.zeros((256, CR, CC), np.float32)
            canvas[:, 1:1 + (b - a), 1:65] = feats[0][img, :, a:b, :]
            mask = np.zeros((CR, CC), np.float32)
            mask[1:1 + (b - a), 1:65] = 1.0
            cores.append((img, ('strip', k, a), canvas, mask))
        canvas = np.zeros((256, CR, CC), np.float32)
        mask = np.zeros((CR, CC), np.float32)
        for (lvl, rr, cc) in PACK:
            s = SIZES[lvl]
            canvas[:, rr:rr + s, cc:cc + s] = feats[lvl][img]
            mask[rr:rr + s, cc:cc + s] = 1.0
        cores.append((img, ('pack',), canvas, mask))
    in_maps = []
    for (img, info, canvas, mask) in cores:
        xin = canvas.reshape(2, 128, CN)
        msk = np.broadcast_to(mask.reshape(1, CN), (128, CN)).copy()
        in_maps.append((img, info, np.ascontiguousarray(xin), msk))
    return in_maps


def _extract_maps(core_specs, results):
    """Reassemble cls logit maps [B,720,H,W] and box maps [B,36,H,W] per level."""
    cls_maps = [np.zeros((2, 720, s, s), np.float32) for s in SIZES]
    box_maps = [np.zeros((2, 36, s, s), np.float32) for s in SIZES]
    for (img, info, _, _), res in zip(core_specs, results):
        co = res['cls_out'].reshape(720, 32, CC)   # [6*120, canvas rows 1..33, 66]
        bo = res['box_out'].reshape(36, 32, CC)
        if info[0] == 'strip':
            _, k, a = info
            r0, r1 = L0_STRIPS[k]
            # image row y lives at canvas row 1 + (y - a) -> co row index (y - a)
            cls_maps[0][img][:, r0:r1, :] = co[:, r0 - a:r1 - a, 1:65]
            box_maps[0][img][:, r0:r1, :] = bo[:, r0 - a:r1 - a, 1:65]
        else:
            for (lvl, rr, cc) in PACK:
                s = SIZES[lvl]
                cls_maps[lvl][img] = co[:, rr - 1:rr - 1 + s, cc:cc + s]
                box_maps[lvl][img] = bo[:, rr - 1:rr - 1 + s, cc:cc + s]
    return cls_maps, box_maps


def kernel(**inputs):
    from concourse.bass_utils import run_bass_kernel_spmd

    inputs = {k: np.asarray(v, np.float32) for k, v in inputs.items()}
    wtr, btr, wcf_, bcf_, wbf_, bbf_ = _prep_weights(inputs)
    core_specs = _make_canvases(inputs)

    if 'nc' not in _COMPILED:
        _COMPILED['nc'] = _build_kernel()
    nc = _COMPILED['nc']

    in_maps = []
    for (img, info, xin, msk) in core_specs:
        in_maps.append(dict(xin=xin, msk=msk, wtr=wtr, btr=btr,
                            wcf=wcf_, bcf=bcf_, wbf=wbf_, bbf=bbf_))
    import time as _time
    _t0 = _time.time()
    res = run_bass_kernel_spmd(nc, in_maps, core_ids=list(range(8)))
    _t1 = _time.time()
    global LAST_EXEC_NS, LAST_RUN_S, LAST_POST_S
    LAST_EXEC_NS = res.exec_time_ns
    LAST_RUN_S = _t1 - _t0
    import time as _time
    _t2 = _time.time()
    cls_maps, box_maps = _extract_maps(core_specs, res.results)
    out = _postprocess(cls_maps, box_maps)
    LAST_POST_S = _time.time() - _t2
    return out


if __name__ == "__main__":
    import pickle
    with open('/tmp/inputs.pkl', 'rb') as f:
        inputs = pickle.load(f)
    out = kernel(**inputs)
    print([o.shape for o in out])
    print(out[0][0, :5])


# revision 7
# speedup vs baseline: 1.3237x; 1.3237x over previous
"""Trainium2 Bass kernel for nn_Model_27685359190754 (RetinaNet-style head + NMS).

Self-contained: conv heads run on 8 NeuronCores (SPMD, data-parallel canvases);
top-k/decode/NMS postprocessing is replicated bitwise-exactly on host.

Sharding: each image's work is split over 4 cores — 3 cores carry 22/21/21-row
strips of the 64x64 level-0 feature map (with conv halos), 1 core carries
levels 1-4 packed into the same canvas geometry. All 8 cores run an identical
program; per-core behavior comes only from input data (canvas + mask).
"""
import numpy as np

# ---------------------------------------------------------------------------
# constants from the model
# ---------------------------------------------------------------------------
STRIDES = [8, 16, 32, 64, 128]
SIZES = [64, 32, 16, 8, 4]
TOP_N = 1000
THRESHOLD = 0.05
NMS_THR = 0.5
DETECTIONS = 100
NA = 9
NC = 80

CELL_ANCHORS = np.array([
 [[-18.0,-8.0,25.0,15.0],[-23.7183,-11.1191,30.7183,18.1191],[-30.9228,-15.0488,37.9228,22.0488],[-12.0,-12.0,19.0,19.0],[-16.1587,-16.1587,23.1587,23.1587],[-21.3984,-21.3984,28.3984,28.3984],[-8.0,-20.0,15.0,27.0],[-11.1191,-26.2381,18.1191,33.2381],[-15.0488,-34.0976,22.0488,41.0976]],
 [[-38.0,-16.0,53.0,31.0],[-49.9564,-22.2381,64.9564,37.2381],[-65.0204,-30.0976,80.0204,45.0976],[-24.0,-24.0,39.0,39.0],[-32.3175,-32.3175,47.3175,47.3175],[-42.7968,-42.7968,57.7968,57.7968],[-14.0,-36.0,29.0,51.0],[-19.7183,-47.4365,34.7183,62.4365],[-26.9228,-61.8456,41.9228,76.8456]],
 [[-74.0,-28.0,105.0,59.0],[-97.3929,-39.4365,128.3929,70.4365],[-126.8661,-53.8456,157.8661,84.8456],[-48.0,-48.0,79.0,79.0],[-64.6349,-64.6349,95.6349,95.6349],[-85.5937,-85.5937,116.5937,116.5937],[-30.0,-76.0,61.0,107.0],[-41.9564,-99.9127,72.9564,130.9127],[-57.0204,-130.0409,88.0204,161.0409]],
 [[-150.0,-60.0,213.0,123.0],[-197.3056,-83.9127,260.3056,146.9127],[-256.907,-114.0409,319.907,177.0409],[-96.0,-96.0,159.0,159.0],[-129.2699,-129.2699,192.2699,192.2699],[-171.1873,-171.1873,234.1873,234.1873],[-58.0,-148.0,121.0,211.0],[-81.3929,-194.7858,144.3929,257.7858],[-110.8661,-253.7322,173.8661,316.7322]],
 [[-298.0,-116.0,425.0,243.0],[-392.0914,-162.7858,519.0914,289.7858],[-510.6392,-221.7322,637.6392,348.7322],[-192.0,-192.0,319.0,319.0],[-258.5398,-258.5398,385.5398,385.5398],[-342.3747,-342.3747,469.3747,469.3747],[-118.0,-300.0,245.0,427.0],[-165.3056,-394.6113,292.3056,521.6113],[-224.907,-513.814,351.907,640.814]]], dtype=np.float32)

# canvas geometry
CR, CC = 34, 66            # canvas rows / cols
CN = CR * CC               # 2244
OUT_ROWS = (1, 33)         # canvas rows where convs are computed
NOUT = (OUT_ROWS[1] - OUT_ROWS[0]) * CC   # 2112 output positions
# trunk/final chunking (rows per matmul chunk)
CHUNKS = [(1, 8), (8, 15), (15, 22), (22, 29), (29, 33)]

# L0 strips (3 per image): output rows
L0_STRIPS = [(0, 22), (22, 43), (43, 64)]
# packed placement for levels 1-4 on the canvas: (level, row0, col0) of the
# *data* cell (top-left), sizes from SIZES.
PACK = [(1, 1, 1), (2, 1, 35), (3, 19, 35), (4, 19, 47)]

_COMPILED = {}
LAST_EXEC_NS = None
LAST_RUN_S = None
LAST_POST_S = None


# ---------------------------------------------------------------------------
# host-side bitwise-exact postprocess (replicates reference decode+NMS,
# validated bitwise against the XLA CPU reference)
# ---------------------------------------------------------------------------
def _fma(a, b, c):
    return (np.asarray(a, np.float64) * np.asarray(b, np.float64)
            + np.asarray(c, np.float64)).astype(np.float32)

_P = [np.float32(v) for v in [1.9875691500E-4, 1.3981999507E-3, 8.3334519073E-3,
                              4.1665795894E-2, 1.6666665459E-1, 5.0000001201E-1]]
_LOG2E = np.float32(1.442695040888963407359924681001892137)
_C1 = np.float32(0.693359375)
_C2 = np.float32(-2.12194440e-4)

def xla_exp(x):
    x = np.asarray(x, np.float32)
    n = np.rint((x * _LOG2E).astype(np.float32)).astype(np.float32)
    r = _fma(n, np.full_like(x, -_C1), x)
    r = _fma(n, np.full_like(x, -_C2), r)
    y = np.full_like(r, _P[0])
    for c in _P[1:]:
        y = _fma(y, r, np.full_like(r, c))
    r2 = (r * r).astype(np.float32)
    y = (_fma(y, r2, r) + np.float32(1.0)).astype(np.float32)
    return np.ldexp(y, n.astype(np.int32)).astype(np.float32)

def xla_sigmoid(x):
    x = np.asarray(x, np.float32)
    e = xla_exp((-x).astype(np.float32))
    return (np.float32(1.0) / (e + np.float32(1.0)).astype(np.float32)).astype(np.float32)

def _delta2box_np(deltas, anchors, stride):
    deltas = deltas.astype(np.float32)
    anchors = anchors.astype(np.float32)
    awh = (anchors[:, 2:] - anchors[:, :2] + np.float32(1.0)).astype(np.float32)
    ctr = (anchors[:, :2] + (np.float32(0.5) * awh).astype(np.float32)).astype(np.float32)
    pctr = ((deltas[:, :2] * awh).astype(np.float32) + ctr).astype(np.float32)
    pwh = (xla_exp(deltas[:, 2:]) * awh).astype(np.float32)
    M = np.float32(511.0)  # w*stride - 1 == 511 for every level
    half = (np.float32(0.5) * pwh).astype(np.float32)
    lo = np.clip((pctr - half).astype(np.float32), np.float32(0.0), M).astype(np.float32)
    hi = np.clip(((pctr + half).astype(np.float32) - np.float32(1.0)).astype(np.float32),
                 np.float32(0.0), M).astype(np.float32)
    return np.concatenate([lo, hi], axis=1).astype(np.float32)

def _decode_level_np(logits, box, stride, anchors):
    AC, H, W = logits.shape
    cm = xla_sigmoid(logits.reshape(-1))
    idx = np.argsort(-cm, kind='stable')[:TOP_N]
    scores = cm[idx]
    cls = (idx // (W * H)) % NC
    x = idx % W
    y = (idx // W) % H
    a = idx // (NC * H * W)
    deltas = box.reshape(NA, 4, H, W)[a, :, y, x]
    grid = (np.stack([x, y, x, y], axis=1).astype(np.float32) * np.float32(stride)
            + anchors[a]).astype(np.float32)
    boxes = _delta2box_np(deltas, grid, stride)
    valid = scores >= np.float32(THRESHOLD)
    return (np.where(valid, scores, np.float32(0.0)).astype(np.float32),
            np.where(valid[:, None], boxes, np.float32(0.0)).astype(np.float32),
            np.where(valid, cls.astype(np.float32), np.float32(0.0)).astype(np.float32))

def _nms_image_np(s, b, c):
    order = np.argsort(-s, kind='stable')
    so, bo, co = s[order], b[order], c[order]
    areas = ((bo[:, 2] - bo[:, 0] + np.float32(1.0)) *
             (bo[:, 3] - bo[:, 1] + np.float32(1.0))).astype(np.float32)
    valid = so > 0
    os_ = np.zeros(DETECTIONS, np.float32)
    ob_ = np.zeros((DETECTIONS, 4), np.float32)
    oc_ = np.zeros(DETECTIONS, np.float32)
    for i in range(DETECTIONS):
        if not valid.any():
            break
        idx = int(np.argmax(valid))
        bi, si, ci, ai = bo[idx], so[idx], co[idx], areas[idx]
        xy1 = np.maximum(bo[:, :2], bi[:2])
        xy2 = np.minimum(bo[:, 2:], bi[2:])
        iw = np.clip((xy2 - xy1 + np.float32(1.0)).astype(np.float32), 0.0, None)
        inter = (iw[:, 0] * iw[:, 1]).astype(np.float32)
        iou = (inter / ((areas + ai).astype(np.float32) - inter).astype(np.float32)).astype(np.float32)
        sup = (so <= si) & (iou > np.float32(NMS_THR)) & (co == ci)
        valid = valid & ~sup
        os_[i] = si
        ob_[i] = bi
        oc_[i] = ci
    return os_, ob_, oc_

def _postprocess(cls_logit_maps, box_maps):
    Bn = cls_logit_maps[0].shape[0]
    out_s = np.zeros((Bn, DETECTIONS), np.float32)
    out_b = np.zeros((Bn, DETECTIONS, 4), np.float32)
    out_c = np.zeros((Bn, DETECTIONS), np.float32)
    for img in range(Bn):
        ss, bs, cs = [], [], []
        for lvl in range(5):
            s, b, c = _decode_level_np(cls_logit_maps[lvl][img], box_maps[lvl][img],
                                       STRIDES[lvl], CELL_ANCHORS[lvl].astype(np.float32))
            ss.append(s); bs.append(b); cs.append(c)
        os_, ob_, oc_ = _nms_image_np(np.concatenate(ss), np.concatenate(bs),
                                      np.concatenate(cs))
        out_s[img] = os_; out_b[img] = ob_; out_c[img] = oc_
    return out_s, out_b, out_c


# ---------------------------------------------------------------------------
# device kernel: generic canvas conv machine
# ---------------------------------------------------------------------------
def _build_kernel():
    import concourse.bass as bass
    import concourse.mybir as mybir
    from concourse import bacc
    from concourse.tile import TileContext

    dt = mybir.dt
    nc = bacc.Bacc()

    xin = nc.dram_tensor("xin", [2, 128, CN], dt.float32, kind="ExternalInput")
    msk = nc.dram_tensor("msk", [128, CN], dt.float32, kind="ExternalInput")
    # trunk weights: [head, layer, tap, cin_half, 128, 256]
    wtr = nc.dram_tensor("wtr", [2, 4, 9, 2, 128, 256], dt.float32, kind="ExternalInput")
    btr = nc.dram_tensor("btr", [2, 4, 256], dt.float32, kind="ExternalInput")
    wcf = nc.dram_tensor("wcf", [9, 2, 128, 720], dt.float32, kind="ExternalInput")
    bcf = nc.dram_tensor("bcf", [720], dt.float32, kind="ExternalInput")
    wbf = nc.dram_tensor("wbf", [9, 2, 128, 36], dt.float32, kind="ExternalInput")
    bbf = nc.dram_tensor("bbf", [36], dt.float32, kind="ExternalInput")
    cls_out = nc.dram_tensor("cls_out", [6, 120, NOUT], dt.float32, kind="ExternalOutput")
    box_out = nc.dram_tensor("box_out", [36, NOUT], dt.float32, kind="ExternalOutput")

    XB = CN + 2  # activation buffer: canvas + 1 pad element each side

    with TileContext(nc) as tc:
        with tc.tile_pool(name="acts", bufs=1) as acts, \
             tc.tile_pool(name="wts", bufs=2) as wts, \
             tc.tile_pool(name="wrnd", bufs=1) as wrnd, \
             tc.tile_pool(name="small", bufs=1) as small, \
             tc.tile_pool(name="stage", bufs=3) as stage, \
             tc.tile_pool(name="psum", bufs=4, space="PSUM") as pp:

            # mask
            mt = small.tile([128, CN], dt.float32, tag="mask")
            nc.sync.dma_start(mt[:], msk[:, :])

            # biases: trunk [128, 16] (head*8 + layer*2 + couthalf)
            btr_t = small.tile([128, 16], dt.float32, tag="btr")
            for h in range(2):
                for l in range(4):
                    for ch in range(2):
                        col = h * 8 + l * 2 + ch
                        nc.sync.dma_start(
                            btr_t[:, col:col + 1],
                            btr[h, l, ch * 128:(ch + 1) * 128].unsqueeze(1))
            bcf_t = small.tile([120, 6], dt.float32, tag="bcf")
            for t in range(6):
                nc.sync.dma_start(bcf_t[:, t:t + 1],
                                  bcf[t * 120:(t + 1) * 120].unsqueeze(1))
            bbf_t = small.tile([36, 1], dt.float32, tag="bbf")
            nc.sync.dma_start(bbf_t[:, 0:1], bbf[:].unsqueeze(1))

            # activation hi/lo canvas pairs: pairs[p] = [(hi0,lo0),(hi1,lo1)]
            pairs = []
            for p in range(2):
                halves = []
                for half in range(2):
                    hi = acts.tile([128, XB], dt.float32r, tag=f"p{p}h{half}hi")
                    lo = acts.tile([128, XB], dt.float32r, tag=f"p{p}h{half}lo")
                    nc.vector.memset(hi[:].bitcast(dt.float32), 0.0)
                    nc.vector.memset(lo[:].bitcast(dt.float32), 0.0)
                    halves.append((hi, lo))
                pairs.append(halves)

            def split_xin(dst_pair):
                """DMA xin and split into (hi, lo) f32r canvases."""
                for half in range(2):
                    hi, lo = dst_pair[half]
                    st = stage.tile([128, CN], dt.float32, tag="xsplit", bufs=1)
                    nc.sync.dma_start(st[:], xin[half])
                    nc.vector.tensor_copy(hi[:, 1:1 + CN], st[:])
                    nc.vector.tensor_sub(st[:], st[:],
                                         hi[:, 1:1 + CN].bitcast(dt.float32))
                    nc.vector.tensor_copy(lo[:, 1:1 + CN], st[:])

            def round_weights(wsrc, nfree):
                """Round a loaded fp32 weight tile into (hi, lo) f32r tiles.
                wsrc is destroyed (holds the lo residual in f32)."""
                w_hi = wrnd.tile([128, 4608], dt.float32r, tag="whi")
                w_lo = wrnd.tile([128, 4608], dt.float32r, tag="wlo")
                # per-tap slices so the first matmuls can start early
                step = 512
                for o in range(0, nfree, step):
                    n = min(step, nfree - o)
                    nc.vector.tensor_copy(w_hi[:, o:o + n], wsrc[:, o:o + n])
                    nc.vector.tensor_sub(wsrc[:, o:o + n], wsrc[:, o:o + n],
                                         w_hi[:, o:o + n].bitcast(dt.float32))
                    nc.vector.tensor_copy(w_lo[:, o:o + n], wsrc[:, o:o + n])
                return w_hi, w_lo

            def mm_group(ps, w_hi, w_lo, src, wof, tw, n, off_fn):
                """54 accumulating split-2 matmuls into ps[:tw, :n].
                wof(tap, cih) -> weight free-dim offset of the [128, tw] slice."""
                k = 0
                for tap in range(9):
                    for cih in range(2):
                        o = wof(tap, cih)
                        xo = off_fn(tap)
                        xhi, xlo = src[cih]
                        for (wt_, xt_) in ((w_hi, xhi), (w_lo, xhi), (w_hi, xlo)):
                            nc.tensor.matmul(
                                ps[:tw, :n],
                                wt_[:, o:o + tw],
                                xt_[:, xo:xo + n],
                                start=(k == 0), stop=(k == 53))
                            k += 1

            def trunk_layer(head, layer, src, dst):
                w = wts.tile([128, 4608], dt.float32, tag="wtr")
                nc.sync.dma_start(
                    w[:].rearrange("p (t c o) -> p t c o", t=9, c=2),
                    wtr[head, layer].transpose([2, 0, 1, 3]))
                w_hi, w_lo = round_weights(w, 4608)
                for (a, b) in CHUNKS:
                    n = (b - a) * CC
                    for coh in range(2):
                        ps = pp.tile([128, 512], dt.float32, tag="ps")
                        mm_group(ps, w_hi, w_lo, src,
                                 lambda tap, cih: (tap * 2 + cih) * 256 + coh * 128,
                                 128, n,
                                 lambda tap: 1 + a * CC + (tap // 3 - 1) * CC + (tap % 3 - 1))
                        st = stage.tile([128, 512], dt.float32, tag="st")
                        bcol = head * 8 + layer * 2 + coh
                        nc.scalar.activation(st[:, :n], ps[:, :n],
                                             mybir.ActivationFunctionType.Relu,
                                             bias=btr_t[:, bcol:bcol + 1])
                        stm = stage.tile([128, 512], dt.float32, tag="stm")
                        nc.vector.tensor_mul(stm[:, :n], st[:, :n],
                                             mt[:, a * CC:a * CC + n])
                        hi, lo = dst[coh]
                        do = 1 + a * CC
                        nc.vector.tensor_copy(hi[:, do:do + n], stm[:, :n])
                        nc.vector.tensor_sub(stm[:, :n], stm[:, :n],
                                             hi[:, do:do + n].bitcast(dt.float32))
                        nc.vector.tensor_copy(lo[:, do:do + n], stm[:, :n])

            def final_layer(head, src):
                if head == 0:
                    # stream 720 couts in 3 groups of 240 (2 tiles of 120)
                    groups = [(g * 240, 2, 120, bcf_t, cls_out) for g in range(3)]
                    wsrc_t = wcf
                else:
                    groups = [(0, 1, 36, bbf_t, box_out)]
                    wsrc_t = wbf
                for (c0, ntile, tw, bias_t, outt) in groups:
                    nf = 9 * 2 * ntile * tw
                    w = wts.tile([128, 4608], dt.float32, tag="wtr")
                    nc.sync.dma_start(
                        w[:, :nf].rearrange("p (t c o) -> p t c o", t=9, c=2),
                        wsrc_t[:, :, :, c0:c0 + ntile * tw].transpose([2, 0, 1, 3]))
                    w_hi, w_lo = round_weights(w, nf)
                    for (a, b) in CHUNKS:
                        n = (b - a) * CC
                        pos0 = (a - 1) * CC
                        for t in range(ntile):
                            gt = c0 // 120 + t  # global couttile idx (cls)
                            ps = pp.tile([128, 512], dt.float32, tag="ps")
                            mm_group(ps, w_hi, w_lo, src,
                                     lambda tap, cih: (tap * 2 + cih) * (ntile * tw) + t * tw,
                                     tw, n,
                                     lambda tap: 1 + a * CC + (tap // 3 - 1) * CC + (tap % 3 - 1))
                            st = stage.tile([128, 512], dt.float32, tag="st")
                            if head == 0:
                                nc.scalar.activation(st[:tw, :n], ps[:tw, :n],
                                                     mybir.ActivationFunctionType.Identity,
                                                     bias=bias_t[:, gt:gt + 1])
                                nc.sync.dma_start(outt[gt, :, pos0:pos0 + n], st[:tw, :n])
                            else:
                                nc.scalar.activation(st[:tw, :n], ps[:tw, :n],
                                                     mybir.ActivationFunctionType.Identity,
                                                     bias=bias_t[:, 0:1])
                                nc.sync.dma_start(outt[:, pos0:pos0 + n], st[:tw, :n])

            for head in range(2):
                split_xin(pairs[head])
                src = pairs[head]
                for layer in range(4):
                    dst = pairs[(head + layer + 1) % 2]
                    trunk_layer(head, layer, src, dst)
                    src = dst
                final_layer(head, src)

    nc.finalize()
    return nc


def _prep_weights(inputs):
    """Host-side weight layout prep: [O,I,3,3] -> [9, 2, 128, O]."""
    def tr(w):
        O = w.shape[0]
        t = np.ascontiguousarray(w.transpose(2, 3, 1, 0).reshape(9, 2, 128, O),
                                 dtype=np.float32)
        return t
    wtr = np.stack([
        np.stack([tr(inputs['cls_w%d' % l]) for l in range(4)]),
        np.stack([tr(inputs['box_w%d' % l]) for l in range(4)]),
    ]).astype(np.float32)
    btr = np.stack([
        np.stack([inputs['cls_b%d' % l] for l in range(4)]),
        np.stack([inputs['box_b%d' % l] for l in range(4)]),
    ]).astype(np.float32)
    return (np.ascontiguousarray(wtr), np.ascontiguousarray(btr),
            tr(inputs['cls_w4']), inputs['cls_b4'].astype(np.float32),
            tr(inputs['box_w4']), inputs['box_b4'].astype(np.float32))


def _make_canvases(inputs):
    """Returns per-core (xin [2,128,CN], msk [128,CN]) and extraction info."""
    feats = [np.asarray(inputs['feat%d' % i], np.float32) for i in range(5)]
    cores = []
    for img in range(2):
        for k in range(3):
            r0, r1 = L0_STRIPS[k]
            a, b = max(0, r0 - 5), min(64, r1 + 5)
            canvas = np.zeros((256, CR, CC), np.float32)
            canvas[:, 1:1 + (b - a), 1:65] = feats[0][img, :, a:b, :]
            mask = np.zeros((CR, CC), np.float32)
            mask[1:1 + (b - a), 1:65] = 1.0
            cores.append((img, ('strip', k, a), canvas, mask))
        canvas = np.zeros((256, CR, CC), np.float32)
        mask = np.zeros((CR, CC), np.float32)
        for (lvl, rr, cc) in PACK:
            s = SIZES[lvl]
            canvas[:, rr:rr + s, cc:cc + s] = feats[lvl][img]
            mask[rr:rr + s, cc:cc + s] = 1.0
        cores.append((img, ('pack',), canvas, mask))
    in_maps = []
    for (img, info, canvas, mask) in cores:
        xin = canvas.reshape(2, 128, CN)
        msk = np.broadcast_to(mask.reshape(1, CN), (128, CN)).copy()
        in_maps.append((img, info, np.ascontiguousarray(xin), msk))
    return in_maps


def _extract_maps(core_specs, results):
    """Reassemble cls logit maps [B,720,H,W] and box maps [B,36,H,W] per level."""
    cls_maps = [np.zeros((2, 720, s, s), np.float32) for s in SIZES]
    box_maps = [np.zeros((2, 36, s, s), np.float32) for s in SIZES]
    for (img, info, _, _), res in zip(core_specs, results):
        co = res['cls_out'].reshape(720, 32, CC)   # [6*120, canvas rows 1..33, 66]
        bo = res['box_out'].reshape(36, 32, CC)
        if info[0] == 'strip':
            _, k, a = info
            r0, r1 = L0_STRIPS[k]
            # image row y lives at canvas row 1 + (y - a) -> co row index (y - a)
            cls_maps[0][img][:, r0:r1, :] = co[:, r0 - a:r1 - a, 1:65]
            box_maps[0][img][:, r0:r1, :] = bo[:, r0 - a:r1 - a, 1:65]
        else:
            for (lvl, rr, cc) in PACK:
                s = SIZES[lvl]
                cls_maps[lvl][img] = co[:, rr - 1:rr - 1 + s, cc:cc + s]
                box_maps[lvl][img] = bo[:, rr - 1:rr - 1 + s, cc:cc + s]
    return cls_maps, box_maps


def kernel(**inputs):
    from concourse.bass_utils import run_bass_kernel_spmd

    inputs = {k: np.asarray(v, np.float32) for k, v in inputs.items()}
    wtr, btr, wcf_, bcf_, wbf_, bbf_ = _prep_weights(inputs)
    core_specs = _make_canvases(inputs)

    if 'nc' not in _COMPILED:
        _COMPILED['nc'] = _build_kernel()
    nc = _COMPILED['nc']

    in_maps = []
    for (img, info, xin, msk) in core_specs:
        in_maps.append(dict(xin=xin, msk=msk, wtr=wtr, btr=btr,
                            wcf=wcf_, bcf=bcf_, wbf=wbf_, bbf=bbf_))
    import time as _time
    _t0 = _time.time()
    res = run_bass_kernel_spmd(nc, in_maps, core_ids=list(range(8)))
    _t1 = _time.time()
    global LAST_EXEC_NS, LAST_RUN_S, LAST_POST_S
    LAST_EXEC_NS = res.exec_time_ns
    LAST_RUN_S = _t1 - _t0
    import time as _time
    _t2 = _time.time()
    cls_maps, box_maps = _extract_maps(core_specs, res.results)
    out = _postprocess(cls_maps, box_maps)
    LAST_POST_S = _time.time() - _t2
    return out


if __name__ == "__main__":
    import pickle
    with open('/tmp/inputs.pkl', 'rb') as f:
        inputs = pickle.load(f)
    out = kernel(**inputs)
    print([o.shape for o in out])
    print(out[0][0, :5])
